# revision 46
# baseline (speedup 1.0000x reference)
"""Trainium2 kernel for nn_HadamardRotation: y = x @ H, H = 4096x4096 Walsh-Hadamard.

Strategy
--------
H4096 = H64 (x) H64 (Kronecker). Writing d = 64*hi + lo, e = 64*hi' + lo':

    y[r, e] = sum_{hi,lo} H64[lo,lo'] * H64[hi,hi'] * x[r, d]

Two matmul stages with 128-wide contraction (block-diagonal I2 (x) H64
weights), separated by an on-chip "corner turn" (SBUF->SBUF DMA partition
shuffle), all operating in the transposed domain (d on partitions, rows on
the free axis). Host does the cheap transposes / index unscrambles; the
device only ever issues contiguous >=1KB DMA lines.

FLOPs: 2 * 128/4096 of the naive matmul = 16x reduction.

Data parallel over 8 cores: rows sharded 16384 -> 8 x 2048, weights
replicated. x and Y travel as bf16 (host casts); tolerance is 2e-2,
measured end-to-end error ~4e-3.

Layouts (per core, R = 2048 rows; DRAM tensors are partition-major so every
DMA descriptor covers a multi-KB contiguous run per partition):
  xt  DRAM in  (128, SLABS, 32, N): xt[64*mu+lo, s, a, rn]
      = x[s*N+rn, 128*a + 64*mu + lo]
  B1  (128,128): B1[64*mu+lo, BETA[c] + 4*(2*nu+mu)] = H64[lo, 2*c+nu]
  B2  (128,128): B2[64*nu+32*mu+a, 2*hi'+nu]         = H64[2*a+mu, hi']
  stage A (chunk a): u_a[p, r] = sum_k B1[k, p] xt[k, s, a, r]
      => u_a[pi(t,c)], pi = BETA[c] + 4t, holds (hi=2a+mu, lo'=2c+nu, t=2nu+mu)
  corner turn (4 src partitions on 4 distinct AXI ports, c in ORD order):
      v_c[32*t + a, r] = u_a[BETA[c] + 4*t, r]
  stage B (chunk c): Y[chat, m, r] = sum_q B2[q, m] v_c[q, r],  c = ORD[chat]
  stage B (chunk c): Y[c, m, r] = sum_q B2[q, m] v_c[q, r]
  Y   DRAM out (128, SLABS, 32, N): Y[2*hi'+nu, s, c, rn]
      = y[s*N+rn, 64*hi' + 2*c + nu]
"""

import math
import numpy as np
import ml_dtypes

import concourse.bass as bass
import concourse.mybir as mybir
import concourse.tile as tile
from concourse import bacc
from concourse.bass_utils import run_bass_kernel_spmd

N_CORES = 8
DIM = 4096
R_TOTAL = 4 * 4096          # rows after flattening (4, 4096, DIM)
R = R_TOTAL // N_CORES      # rows per core
N = 512                     # free-dim slab (one PSUM bank of fp32)
SLABS = R // N

# dtype mode: "fp32" (exact, PE 4 cyc/row), "fp32r" (fp32 storage, fast PE
# mode), "bf16" (half storage+DMA for x/intermediate, exact weights)
MODE = "bf16"

# tuning knobs (overridable for benching)
CFG = dict(
    ycopy="rr",        # engine for psum->sbuf copy of stage-B out: vector|scalar|rr
    ucopy="rr",        # engine for psum->sbuf copy of stage-A out
    turn_eng="rr3",    # corner-turn DMA engine: scalar|sync|gpsimd|rr2|rr3
    in_eng="sync",
    out_eng="sync",
    in_batch=16,       # chunks per input DMA
    out_batch=8,       # batch output DMAs over this many c-chunks
    turn_slabs=2,      # how many N-slabs share one corner-turn DMA
    pipeline=1,        # emit stage A of group sg+1 before stage B of sg
    ybf16=1,           # stage-B out written to DRAM as bf16; host upcasts
    xbufs=2, ubufs=2, vbufs=6, ybufs=2,
)


def _walsh_hadamard64():
    h = np.array([[1.0]], dtype=np.float64)
    while h.shape[0] < 64:
        h = np.block([[h, h], [h, -h]]) / math.sqrt(2.0)
    return h.astype(np.float32)


# u partition layout: chunk c's four lanes (t = 2nu+mu) live on partitions
# pi(t,c) = 16*(c//4) + (c%4) + 4*t.  SBUF AXI port(p) = 2*((p//4)%8)+p//64,
# so each corner turn's 4 source partitions land on 4 DISTINCT ports (vs 1-2
# for naive layouts; HW-measured 43-64 GB/s there).  The c iteration order
# ORD rotates through disjoint port quartets so in-flight turns cover all 16.
BETA = [16 * (c // 4) + (c % 4) for c in range(32)]
B_ORDER = [0, 1, 4, 5, 2, 3, 6, 7]
ORD = [4 * B_ORDER[i % 8] + (i // 8) for i in range(32)]


def _build_weights(H64):
    B1 = np.zeros((128, 128), dtype=np.float32)
    for mu in range(2):
        for nu in range(2):
            for c in range(32):
                B1[64 * mu:64 * (mu + 1),
                   BETA[c] + 4 * (2 * nu + mu)] = H64[:, 2 * c + nu]
    B2 = np.zeros((128, 128), dtype=np.float32)
    b2v = B2.reshape(2, 2, 32, 64, 2)
    for nu in range(2):
        for mu in range(2):
            b2v[nu, mu, :, :, nu] = H64[mu::2, :]
    return B1, B2


_NC_CACHE = {}


def _build_bass(mode, loop=0, cfg=None):
    cfg = dict(CFG, **(cfg or {}))
    key = (mode, loop, tuple(sorted(cfg.items())))
    if key in _NC_CACHE:
        return _NC_CACHE[key]

    f32 = mybir.dt.float32
    bf16 = mybir.dt.bfloat16
    dt_in = bf16 if mode == "bf16" else f32
    dt_y = bf16 if cfg["ybf16"] else f32
    mm_cast = (lambda ap: ap.bitcast(mybir.dt.float32r)) if mode == "fp32r" else (lambda ap: ap)

    nc = bacc.Bacc("TRN2", target_bir_lowering=False, debug=False,
                   num_devices=N_CORES)
    xt_d = nc.dram_tensor("xt", [128, SLABS, 32, N], dt_in, kind="ExternalInput")
    B1_d = nc.dram_tensor("B1", [128, 128], dt_in, kind="ExternalInput")
    B2_d = nc.dram_tensor("B2", [128, 128], dt_in, kind="ExternalInput")
    Y_d = nc.dram_tensor("Y", [128, SLABS, 32, N], dt_y, kind="ExternalOutput")

    IB = cfg["in_batch"]
    OB = cfg["out_batch"]
    TS = cfg["turn_slabs"]
    L = TS * N

    with tile.TileContext(nc) as tc:
        with (
            tc.tile_pool(name="wpool", bufs=1) as wpool,
            tc.tile_pool(name="xpool", bufs=cfg["xbufs"]) as xpool,
            tc.tile_pool(name="upool", bufs=cfg["ubufs"]) as upool,
            tc.tile_pool(name="vpool", bufs=cfg["vbufs"]) as vpool,
            tc.tile_pool(name="ypool", bufs=cfg["ybufs"]) as ypool,
            tc.tile_pool(name="psA", bufs=4, space="PSUM") as psA,
            tc.tile_pool(name="psB", bufs=4, space="PSUM") as psB,
        ):
            B1_sb = wpool.tile([128, 128], dt_in)
            nc.sync.dma_start(B1_sb[:], B1_d[:])
            B2_sb = wpool.tile([128, 128], dt_in)
            nc.sync.dma_start(B2_sb[:], B2_d[:])

            in_eng = getattr(nc, cfg["in_eng"])
            out_eng = getattr(nc, cfg["out_eng"])
            if cfg["turn_eng"] == "rr2":
                turn_rr = [nc.scalar, nc.gpsimd]
            elif cfg["turn_eng"] == "rr3":
                turn_rr = [nc.scalar, nc.gpsimd, nc.sync]
            else:
                turn_rr = [getattr(nc, cfg["turn_eng"])]

            cnt = [0]

            def copy(kind, dst, src):
                i = cnt[0]
                cnt[0] += 1
                if kind == "vector":
                    nc.vector.tensor_copy(dst, src)
                elif kind == "scalar":
                    nc.scalar.copy(dst, src)
                elif kind == "rr":
                    if i % 2 == 0:
                        nc.vector.tensor_copy(dst, src)
                    else:
                        nc.scalar.copy(dst, src)
                else:
                    nc.any.tensor_copy(dst, src)

            def phaseA(sg):
                u_all = upool.tile([128, 32, L], dt_in)
                for ts in range(TS):
                    s = sg * TS + ts
                    for g in range(32 // IB):
                        xg = xpool.tile([128, IB, N], dt_in)
                        in_eng.dma_start(
                            xg[:], xt_d[:, s, IB * g:IB * (g + 1), :])
                        for j in range(IB):
                            a = IB * g + j
                            pu = psA.tile([128, N], f32)
                            nc.tensor.matmul(pu[:], mm_cast(B1_sb[:]),
                                             mm_cast(xg[:, j, :]),
                                             start=True, stop=True)
                            copy(cfg["ucopy"],
                                 u_all[:, a, ts * N:(ts + 1) * N], pu[:])
                return u_all

            def phaseB(sg, u_all):
                ut = u_all.tensor
                PU = u_all.ap[0][0]  # partition stride in elements
                for cb in range(32 // OB):
                    ybs = [ypool.tile([128, OB, N], dt_y, name=f"yb{ts}")
                           for ts in range(TS)]
                    for j in range(OB):
                        chat = cb * OB + j
                        c = ORD[chat]
                        vc = vpool.tile([128, L], dt_in)
                        # v_c[32*t + a, r] = u[pi(t,c), a, r]
                        in_ap = bass.AP(ut, BETA[c] * PU,
                                        [[4 * PU, 4], [L, 32], [1, L]])
                        turn_rr[chat % len(turn_rr)].dma_start(vc[:], in_ap)
                        for ts in range(TS):
                            py = psB.tile([128, N], f32)
                            nc.tensor.matmul(py[:], mm_cast(B2_sb[:]),
                                             mm_cast(vc[:, ts * N:(ts + 1) * N]),
                                             start=True, stop=True)
                            copy(cfg["ycopy"], ybs[ts][:, j, :], py[:])
                    for ts in range(TS):
                        s = sg * TS + ts
                        out_eng.dma_start(
                            Y_d[:, s, cb * OB:(cb + 1) * OB, :], ybs[ts][:])

            def body():
                if cfg["pipeline"]:
                    # software pipeline: emit stage A of group sg+1 before
                    # stage B of sg, so PE never stalls on the turn.
                    pending = None
                    for sg in range(SLABS // TS):
                        u_all = phaseA(sg)
                        if pending is not None:
                            phaseB(*pending)
                        pending = (sg, u_all)
                    phaseB(*pending)
                else:
                    for sg in range(SLABS // TS):
                        phaseB(sg, phaseA(sg))

            if loop:
                with tc.For_i(0, loop, 1):
                    body()
            else:
                body()

    nc.compile()
    _NC_CACHE[key] = nc
    return nc


def _prep_inputs(x, H, mode, cfg=None):
    cfg = dict(CFG, **(cfg or {}))
    np_in = ml_dtypes.bfloat16 if mode == "bf16" else np.float32
    H64 = (np.asarray(H, dtype=np.float32)[::64, ::64] * 8.0).astype(np.float32)
    B1, B2 = _build_weights(H64)
    B1 = B1.astype(np_in)
    B2 = B2.astype(np_in)
    xf = np.asarray(x, dtype=np.float32).reshape(R_TOTAL, DIM)
    in_maps = []
    for i in range(N_CORES):
        shard = xf[i * R:(i + 1) * R]                     # (R, DIM)
        # xt[k, s, a, rn] = x[s*N + rn, 128*a + k]
        xt = shard.reshape(SLABS, N, 32, 128).transpose(3, 0, 2, 1)
        xt = np.ascontiguousarray(xt, dtype=np_in)
        in_maps.append({"xt": xt, "B1": B1, "B2": B2})
    return in_maps


def _unscramble(results):
    iord = np.argsort(np.asarray(ORD))           # iord[c] = position of c in ORD
    outs = []
    for i in range(N_CORES):
        Y = np.asarray(results[i]["Y"])          # (128, SLABS, 32, N)
        # Y[2*hi'+nu, s, chat, rn] = y[s*N + rn, 64*hi' + 2*ORD[chat] + nu]
        y = (Y.reshape(64, 2, SLABS, 32, N)[:, :, :, iord, :]
             .transpose(2, 4, 0, 3, 1).reshape(R, DIM))
        outs.append(y.astype(np.float32))
    return np.concatenate(outs, axis=0).reshape(4, 4096, DIM)


def kernel(x, H, _trace=False, _cfg=None):
    nc = _build_bass(MODE, cfg=_cfg)
    in_maps = _prep_inputs(x, H, MODE, cfg=_cfg)
    res = run_bass_kernel_spmd(nc, in_maps, core_ids=list(range(N_CORES)),
                               trace=_trace)
    out = _unscramble(res.results)
    if _trace:
        return out, res
    return out
